# revision 11
# baseline (speedup 1.0000x reference)
"""Trainium2 Bass kernel for the LVIS-style masked sigmoid-BCE loss.

loss = sum(wm * (softplus(x) - x * onehot(labels))) / n_i over
x [16384, 1231].  Structure exploited (true for the reference
generator): fg rows have u==0 (need only the thresholded softplus sum
over all columns); bg rows have fg=0 (need only plain softplus sums
over their selected per-class column blocks, contiguous after a host
column permutation [freq | common | rare]).

Identities used (all sums per 128-row slot, f32 psum / f32 accum):
  fg:  sum_j c*softplus(x) = sum(mx) + (THR+rT)*sum(c) + a1*sum(w) + K*N
       with mx = max(x,THR), c = (x>=THR), w = e^-mx = min(e^-x, e^-THR),
       using a deg-1 fit of ln(1+w) on [0, e^-THR] exact at e^-THR, so
       pad rows (x=-30) contribute exactly zero.
  bg:  sum_blk softplus = sum(relu(x)) + d1*sum(eta) + d0*N_blk
       with eta = e^-|x| (|x| via uint16 sign-strip), deg-1 fit of
       ln(1+eta) on [0,1].

Engine assignment (per core; measured rates ACT 0.95ns/el, DVE ts 4x
0.26, tt 2x 0.52, PE 0.42-0.52/row):
  ACT: one Exp pass over fg (9848 cols/lane) + bg (3933), with free
       accumulate row-sums providing sum(w) and sum(eta).
  DVE: non-accumulate 4x tensor_scalar ops only (mx, c, relu, |x|);
       accumulating tensor_scalar runs at 1x so it is avoided for all
       large tiles (only tiny blob/grid work uses it).
  PE:  sum(mx), sum(c), sum(relu) via ones/indicator-stationary
       matmuls into [1,512] psums (later matmuls clipped into the
       region zeroed by the first).
All per-row accumulator columns gather in one [128, NG] grid reduced
by a single ones-matmul + coefficient dot.
"""

import math
from contextlib import ExitStack

import numpy as np
import ml_dtypes

import concourse.bass as bass
import concourse.tile as tile
from concourse import bacc, mybir
from concourse.bass_utils import run_bass_kernel_spmd

N_I, N_C = 16384, 1231
N_CORES = 8
N_LOC = N_I // N_CORES
P = 128
NSLOT = N_LOC // P
THR = float(math.log(0.7 / 0.3))
ETA_T = float(math.exp(-THR))
R_T = float(math.log1p(ETA_T))
# deg-1 minimax fit of ln(1+w) on [0, ETA_T] constrained exact at ETA_T
A1 = 0.80735
A0 = R_T - A1 * ETA_T
# deg-1 minimax fit of ln(1+eta) on [0, 1]
D0, D1 = 0.02984, 0.6931
CJ = THR + R_T                      # coefficient of sum(c)

F32 = mybir.dt.float32
BF16 = mybir.dt.bfloat16
I32 = mybir.dt.int32
U16 = mybir.dt.uint16
AF = mybir.ActivationFunctionType
OP = mybir.AluOpType
PAD_X = -30.0


def _sizes2(n):
    if n <= 0:
        return []
    out = []
    rem = n
    while rem > 0:
        sz = min(2, rem)
        out.append(sz)
        rem -= sz
    return out


def _sizes(n, pref):
    if n <= 0:
        return []
    out = []
    first = True
    rem = n
    while rem > 0:
        s = 1 if (first and rem > 2) else min(pref, rem)
        out.append(s)
        rem -= s
        first = False
    return out


def _chunks(n, w):
    return [(c0, min(c0 + w, n)) for c0 in range(0, n, w)]


def _build_nc(cfg):
    NFG, NB4, NLAST, F, C, R, EXTB = cfg
    EXT = N_C
    FG_SIZES = _sizes(NFG, 2)
    B4_SIZES = [NB4] if NB4 else []
    NFGI = len(FG_SIZES)
    NB4I = len(B4_SIZES)
    NGCOL = NSLOT + 1
    # G grid columns
    iW = 0                              # NFGI fg sum(w) accums
    iBW = iW + NFGI                     # blob sum(w)
    iBMX = iBW + 1                      # blob sum(mx)
    iBC = iBMX + 1                      # blob sum(c)
    iCC = iBC + 1                       # NFGI cache-summed sum(c) (groups>=CPE)
    iH4 = iCC + NFGI                    # b4 sum(eta) accum
    iR4 = iH4 + max(NB4I, 1)            # b4 sum(relu) cache accum
    iHL = iR4 + max(NB4I, 1)            # 3*NLAST last sum(eta), lind-weighted
    iLRL = iHL + 3 * NLAST              # 3*NLAST last sum(relu), lind-weighted
    iRR = iLRL + 3 * NLAST              # per-row corrections
    NG = iRR + 1

    nc = bacc.Bacc(None, target_bir_lowering=False)
    x_fg_d = nc.dram_tensor("x_fg", [NFG * P, EXT], BF16, kind="ExternalInput")
    x_b4_d = nc.dram_tensor("x_b4", [max(NB4, 1) * P, max(F, 1)], BF16,
                            kind="ExternalInput")
    x_la_d = nc.dram_tensor("x_la", [NLAST * P, EXT], BF16, kind="ExternalInput")
    x_eb_d = nc.dram_tensor("x_eb", [P, EXTB], BF16, kind="ExternalInput")
    g_d = nc.dram_tensor("g", [P, NGCOL], BF16, kind="ExternalInput")
    wa_d = nc.dram_tensor("wa", [P, NGCOL], F32, kind="ExternalInput")
    wb_d = nc.dram_tensor("wb", [P, NGCOL], F32, kind="ExternalInput")
    wg_d = nc.dram_tensor("wg", [P, NGCOL], F32, kind="ExternalInput")
    lind_d = nc.dram_tensor("lind", [P, 3 * NLAST], BF16, kind="ExternalInput")
    coef_d = nc.dram_tensor("coef", [1, NG], F32, kind="ExternalInput")
    out_d = nc.dram_tensor("out", [1, 1], F32, kind="ExternalOutput")

    xfg = x_fg_d.rearrange("(k p) c -> p k c", p=P)
    xb4 = x_b4_d.rearrange("(k p) c -> p k c", p=P)
    xla = x_la_d.rearrange("(k p) c -> p k c", p=P)

    FG_STARTS = [sum(FG_SIZES[:i]) for i in range(NFGI)]
    B4_STARTS = [sum(B4_SIZES[:i]) for i in range(NB4I)]
    FG_CH = _chunks(EXT, 512)
    BLK = [(0, F), (F, F + C), (F + C, F + C + R)]
    CPE = 2   # fg groups whose sum(c) goes to PE; the rest use cache accums

    with tile.TileContext(nc) as tc, ExitStack() as ctx:
        const = ctx.enter_context(tc.tile_pool(name="const", bufs=1))
        xpool = ctx.enter_context(tc.tile_pool(name="x", bufs=1))
        mpool = ctx.enter_context(tc.tile_pool(name="m", bufs=1))
        epool = ctx.enter_context(tc.tile_pool(name="e", bufs=1))
        spool = ctx.enter_context(tc.tile_pool(name="s", bufs=1))
        fin = ctx.enter_context(tc.tile_pool(name="fin", bufs=1))
        psum = ctx.enter_context(tc.tile_pool(name="psum", bufs=1, space="PSUM"))

        # ---- tiny consts first (one DMA burst on the sync queue), then
        # the fg x stream; bulk bg x on the gpsimd queue
        g_sb = const.tile([P, NGCOL], BF16)
        nc.sync.dma_start(g_sb[:], g_d[:])
        wa_sb = const.tile([P, NGCOL], F32)
        nc.sync.dma_start(wa_sb[:], wa_d[:])
        wb_sb = const.tile([P, NGCOL], F32)
        nc.sync.dma_start(wb_sb[:], wb_d[:])
        wg_sb = const.tile([P, NGCOL], F32)
        nc.sync.dma_start(wg_sb[:], wg_d[:])
        lind_sb = const.tile([P, 3 * NLAST], BF16)
        nc.sync.dma_start(lind_sb[:], lind_d[:])
        coef_sb = const.tile([1, NG], F32)
        nc.sync.dma_start(coef_sb[:], coef_d[:])

        xfg_t = [None] * NFGI
        for i, (k0, s) in enumerate(zip(FG_STARTS, FG_SIZES)):
            xfg_t[i] = xpool.tile([P, s, EXT], BF16, tag="xfg", name=f"xfg{i}",
                                  bufs=NFGI)
            nc.sync.dma_start(xfg_t[i][:], xfg[:, k0:k0 + s, :])
        ones_bf = const.tile([P, 1], BF16)
        nc.vector.memset(ones_bf[:], 1.0)
        ones_f = const.tile([P, 1], F32)
        nc.vector.memset(ones_f[:], 1.0)

        xeb_t = xpool.tile([P, EXTB], BF16, name="xeb")
        nc.gpsimd.dma_start(xeb_t[:], x_eb_d[:])
        xla_t = [None] * NLAST
        for k in range(NLAST):
            xla_t[k] = xpool.tile([P, EXT], BF16, tag="xla", name=f"xla{k}",
                                  bufs=max(NLAST, 1))
            nc.gpsimd.dma_start(xla_t[k][:], xla[:, k, :])
        xb4_t = [None] * NB4I
        for i, (k0, s) in enumerate(zip(B4_STARTS, B4_SIZES)):
            xb4_t[i] = xpool.tile([P, s, F], BF16, tag="xb4", name=f"xb4{i}",
                                  bufs=max(NB4I, 1))
            nc.gpsimd.dma_start(xb4_t[i][:], xb4[:, k0:k0 + s, :])


        # ---- grids and psums
        G = fin.tile([P, NG], F32)
        nc.vector.memset(G[:], 0.0)
        LH = fin.tile([P, 3 * NLAST], F32)     # raw LAST eta accums
        LRL = fin.tile([P, 3 * NLAST], F32)    # raw LAST relu accums
        P_mx = psum.tile([1, 512], F32, space="PSUM")
        P_c = psum.tile([1, 512], F32, space="PSUM")
        PG = psum.tile([1, NG], F32, space="PSUM")

        # ---- ACT warmup (hoists the exp table load)
        warm = fin.tile([1, 2], F32)
        nc.vector.memset(warm[:], 0.0)
        warm_o = fin.tile([1, 2], F32)
        act_order = [nc.scalar.activation(warm_o[:], warm[:], AF.Exp)]

        CPE_SLOTS = sum(FG_SIZES[:CPE])
        mm_mx = [0]
        n_mm_mx = NFG * len(FG_CH)
        mm_c = [0]
        n_mm_c = CPE_SLOTS * len(FG_CH)

        def fg_mm(psum_t, src, j, ctr, nmm):
            for (c0, c1) in FG_CH:
                w = c1 - c0
                nc.tensor.matmul(psum_t[0:1, 0:w], ones_bf[:], src[:, j, c0:c1],
                                 start=(ctr[0] == 0),
                                 stop=(ctr[0] == nmm - 1),
                                 skip_group_check=True)
                ctr[0] += 1

        # ---- FG slots
        eta_fg = [None] * NFGI
        for i, s in enumerate(FG_SIZES):
            mx = mpool.tile([P, s, EXT], BF16, tag="mx", name=f"mx{i}", bufs=3)
            nc.vector.tensor_scalar(mx[:], xfg_t[i][:], THR, None, OP.max)
            csc = spool.tile([P, s, EXT], BF16, tag="csc", name=f"c{i}", bufs=2)
            if i < CPE:
                nc.vector.tensor_scalar(csc[:], xfg_t[i][:], THR, None,
                                        OP.is_ge)
            else:
                nc.vector.tensor_scalar(csc[:], xfg_t[i][:], THR, 0.0,
                                        OP.is_ge, op1=OP.add,
                                        accum_out=G[:, iCC + i:iCC + i + 1])
            eta_fg[i] = epool.tile([P, s, EXT], BF16, tag="eta", name=f"eta{i}",
                                   bufs=2)
            act_order.append(nc.scalar.activation(
                eta_fg[i][:], mx[:], AF.Exp, scale=-1.0,
                accum_out=G[:, iW + i:iW + i + 1]))
            for j in range(s):
                fg_mm(P_mx, mx, j, mm_mx, n_mm_mx)
                if i < CPE:
                    fg_mm(P_c, csc, j, mm_c, n_mm_c)
            if i == 1:
                # blob: overflow fg rows on a flat [P, EXTB] tile (tiny ops)
                mxe = mpool.tile([P, EXTB], BF16, name="mxe")
                nc.vector.tensor_scalar(mxe[:], xeb_t[:], THR, 0.0, OP.max,
                                        op1=OP.add,
                                        accum_out=G[:, iBMX:iBMX + 1])
                ce = spool.tile([P, EXTB], BF16, name="ce")
                nc.vector.tensor_scalar(ce[:], xeb_t[:], THR, 0.0, OP.is_ge,
                                        op1=OP.add, accum_out=G[:, iBC:iBC + 1])
                etae = epool.tile([P, EXTB], BF16, name="etae")
                act_order.append(nc.scalar.activation(
                    etae[:], mxe[:], AF.Exp, scale=-1.0,
                    accum_out=G[:, iBW:iBW + 1]))

        # ---- B4 slots: relu + eta, freq block only
        for i, s in enumerate(B4_SIZES):
            z = mpool.tile([P, s, F], BF16, tag="z4", name=f"z4{i}", bufs=2)
            nc.vector.tensor_scalar(z[:].bitcast(U16), xb4_t[i][:].bitcast(U16),
                                    0x7FFF, None, OP.bitwise_and)
            rl = spool.tile([P, s, F], BF16, tag="rl4", name=f"rl4{i}", bufs=2)
            nc.vector.tensor_scalar(rl[:], xb4_t[i][:], 0.0, 0.0, OP.max,
                                    op1=OP.add,
                                    accum_out=G[:, iR4 + i:iR4 + i + 1])
            eta_b = epool.tile([P, s, F], BF16, tag="eta4", name=f"eta4{i}",
                               bufs=2)
            act_order.append(nc.scalar.activation(
                eta_b[:], z[:], AF.Exp, scale=-1.0,
                accum_out=G[:, iH4 + i:iH4 + i + 1]))

        # ---- per-row corrections (gathered g), softplus via the bg poly
        g32 = fin.tile([P, NGCOL], F32)
        nc.vector.tensor_copy(g32[:], g_sb[:])
        zg = fin.tile([P, NGCOL], BF16)
        nc.vector.tensor_scalar(zg[:].bitcast(U16), g_sb[:].bitcast(U16),
                                0x7FFF, None, OP.bitwise_and)
        eta_g = fin.tile([P, NGCOL], F32)
        act_order.append(nc.scalar.activation(eta_g[:], zg[:], AF.Exp,
                                              scale=-1.0))

        # ---- LAST slots: full width, per-block
        for k in range(NLAST):
            zl = mpool.tile([P, EXT], BF16, tag="zl", name=f"zl{k}", bufs=2)
            nc.vector.tensor_scalar(zl[:].bitcast(U16), xla_t[k][:].bitcast(U16),
                                    0x7FFF, None, OP.bitwise_and)
            rll = spool.tile([P, EXT], BF16, tag="rll", name=f"rll{k}", bufs=2)
            eta_l = epool.tile([P, EXT], BF16, tag="etal", name=f"etal{k}",
                               bufs=2)
            for b, (c0, c1) in enumerate(BLK):
                if c1 <= c0:
                    continue
                nc.vector.tensor_scalar(
                    rll[:, c0:c1], xla_t[k][:, c0:c1], 0.0, 0.0, OP.max,
                    op1=OP.add,
                    accum_out=LRL[:, 3 * k + b:3 * k + b + 1])
                act_order.append(nc.scalar.activation(
                    eta_l[:, c0:c1], zl[:, c0:c1], AF.Exp, scale=-1.0,
                    accum_out=LH[:, 3 * k + b:3 * k + b + 1]))

        # finish the correction chain (f32, tiny)
        rlg = fin.tile([P, NGCOL], F32)
        nc.vector.tensor_scalar(rlg[:], g32[:], 0.0, None, OP.max)
        sp1 = fin.tile([P, NGCOL], F32)
        nc.vector.tensor_scalar(sp1[:], eta_g[:], D1, D0, OP.mult, op1=OP.add)
        spg = fin.tile([P, NGCOL], F32)
        nc.vector.tensor_tensor(spg[:], sp1[:], rlg[:], OP.add)
        mlt = fin.tile([P, NGCOL], F32)
        nc.vector.tensor_scalar(mlt[:], g32[:], THR, None, OP.is_lt)
        w1 = fin.tile([P, NGCOL], F32)
        nc.vector.tensor_tensor(w1[:], mlt[:], wb_sb[:], OP.mult)
        w2 = fin.tile([P, NGCOL], F32)
        nc.vector.tensor_tensor(w2[:], w1[:], wa_sb[:], OP.add)
        t4t = fin.tile([P, NGCOL], F32)
        nc.vector.tensor_tensor(t4t[:], w2[:], spg[:], OP.mult)
        gw = fin.tile([P, NGCOL], F32)
        nc.vector.tensor_tensor(gw[:], g32[:], wg_sb[:], OP.mult)
        t5 = fin.tile([P, NGCOL], F32)
        nc.vector.tensor_tensor(t5[:], t4t[:], gw[:], OP.subtract)
        nc.vector.reduce_sum(G[:, iRR:iRR + 1], t5[:], axis=mybir.AxisListType.X)

        # LAST accums weighted by per-row block indicators
        for k in range(NLAST):
            for b in range(3):
                col = 3 * k + b
                nc.vector.tensor_tensor(G[:, iHL + col:iHL + col + 1],
                                        LH[:, col:col + 1],
                                        lind_sb[:, col:col + 1], OP.mult)
                nc.vector.tensor_tensor(G[:, iLRL + col:iLRL + col + 1],
                                        LRL[:, col:col + 1],
                                        lind_sb[:, col:col + 1], OP.mult)

        # ---- epilogue
        nc.tensor.matmul(PG[0:1, :], ones_f[:], G[:], start=True, stop=True,
                         skip_group_check=True)
        pgc = fin.tile([1, NG], F32)
        nc.vector.tensor_copy(pgc[:], PG[:])
        pgw = fin.tile([1, NG], F32)
        nc.vector.tensor_tensor(pgw[:], pgc[:], coef_sb[:], OP.mult)
        s1 = fin.tile([1, 1], F32)
        nc.vector.reduce_sum(s1[:], pgw[:], axis=mybir.AxisListType.X)

        smx = fin.tile([1, 1], F32)
        nc.vector.reduce_sum(smx[:], P_mx[:], axis=mybir.AxisListType.X)
        sc = fin.tile([1, 1], F32)
        nc.vector.reduce_sum(sc[:], P_c[:], axis=mybir.AxisListType.X)
        o1 = fin.tile([1, 1], F32)
        nc.vector.tensor_scalar(o1[:], sc[:], CJ, None, OP.mult)
        o2 = fin.tile([1, 1], F32)
        nc.vector.tensor_tensor(o2[:], smx[:], o1[:], OP.add)
        out_sb = fin.tile([1, 1], F32)
        nc.vector.tensor_tensor(out_sb[:], o2[:], s1[:], OP.add)
        nc.sync.dma_start(out_d[:], out_sb[:])

        for prev, nxt in zip(act_order, act_order[1:]):
            tile.add_dep_helper(nxt.ins, prev.ins, sync=False,
                                reason="ACT stream order")

    nc.finalize()
    return nc


_NC_CACHE = {}


def _get_nc(cfg):
    if cfg not in _NC_CACHE:
        _NC_CACHE[cfg] = _build_nc(cfg)
    return _NC_CACHE[cfg]


def _coef_vec(cfg):
    NFG, NB4, NLAST, F, C, R, EXTB = cfg
    NFGI = len(_sizes(NFG, 2))
    NB4I = 1 if NB4 else 0
    iW = 0
    iBW = iW + NFGI
    iBMX = iBW + 1
    iBC = iBMX + 1
    iCC = iBC + 1
    iH4 = iCC + NFGI
    iR4 = iH4 + max(NB4I, 1)
    iHL = iR4 + max(NB4I, 1)
    iLRL = iHL + 3 * NLAST
    iRR = iLRL + 3 * NLAST
    NG = iRR + 1
    coef = np.zeros((1, NG), np.float32)
    coef[0, iW:iW + NFGI] = A1
    coef[0, iBW] = A1
    coef[0, iBMX] = 1.0
    coef[0, iBC] = CJ
    coef[0, iCC:iCC + NFGI] = CJ
    coef[0, iH4:iH4 + NB4I] = D1
    coef[0, iR4:iR4 + NB4I] = 1.0
    coef[0, iHL:iHL + 3 * NLAST] = D1
    coef[0, iLRL:iLRL + 3 * NLAST] = 1.0
    coef[0, iRR] = 1.0
    return coef


def _prep(cls_logits, labels, rare_mask, common_mask, freq_mask,
          rare_sel, common_sel, freq_sel):
    lab = np.asarray(labels).astype(np.int64)
    rm = np.asarray(rare_mask).astype(np.float32)
    cm = np.asarray(common_mask).astype(np.float32)
    fm = np.asarray(freq_mask).astype(np.float32)
    rs = np.asarray(rare_sel).astype(np.int64)
    cs = np.asarray(common_sel).astype(np.int64)
    fs = np.asarray(freq_sel).astype(np.int64)

    t = rs + 2 * cs + 4 * fs
    fg = lab != 0
    if np.any(fg & (t > 0)):
        return None
    fmb, cmb, rmb = fm > 0, cm > 0, rm > 0
    if np.any((fmb & cmb) | (fmb & rmb) | (cmb & rmb)):
        return None
    bg_t = t[~fg]
    if np.any((bg_t > 0) & (bg_t < 4)):
        # bg rows without the freq bit break the shared relu-psum layout
        return None
    fcols = np.nonzero(fmb)[0]
    ccols = np.nonzero(cmb)[0]
    rcols = np.nonzero(rmb)[0]
    ocols = np.nonzero(~(fmb | cmb | rmb))[0]
    F, C, R = len(fcols), len(ccols), len(rcols)
    if F > 512 or C > 512 or R > 512 or F < 1:
        return None
    perm = np.concatenate([fcols, ccols, rcols, ocols])
    inv = np.empty(N_C, np.int64)
    inv[perm] = np.arange(N_C)
    labp = inv[lab]

    x = np.asarray(cls_logits, dtype=np.float32)[:, perm]
    xb = np.ascontiguousarray(x).astype(ml_dtypes.bfloat16)

    u8 = np.zeros((8, N_C), np.float32)
    for tt_ in range(8):
        m = np.zeros(N_C, np.float32)
        if tt_ & 1:
            m = np.maximum(m, rm)
        if tt_ & 2:
            m = np.maximum(m, cm)
        if tt_ & 4:
            m = np.maximum(m, fm)
        u8[tt_] = m
    h = u8[t, lab]
    fgf = fg.astype(np.float32)
    wa_all = (1.0 - h) * (1.0 - fgf)
    wb_all = (1.0 - h) * fgf

    idx_fg = np.nonzero(fg)[0]
    idx_b4 = np.nonzero((~fg) & (t == 4))[0]
    idx_la = np.nonzero((~fg) & (t != 4))[0]
    cores_fg = [idx_fg[c::N_CORES] for c in range(N_CORES)]
    cores_b4 = [idx_b4[c::N_CORES] for c in range(N_CORES)]
    cores_la = [idx_la[c::N_CORES] for c in range(N_CORES)]

    min_fg = min(len(v) for v in cores_fg)
    min_b4 = min(len(v) for v in cores_b4)
    NFG = min(8, min_fg // P)
    if NFG < 1:
        return None
    NB4 = max(0, min(NSLOT - NFG - 1, min_b4 // P))
    NLAST = NSLOT - NFG - NB4
    max_blob = max(len(v) for v in cores_fg) - NFG * P
    for c in range(N_CORES):
        n_last_rows = (len(cores_b4[c]) - min(len(cores_b4[c]), NB4 * P)
                       + len(cores_la[c]))
        if n_last_rows > NLAST * P:
            return None
    if max_blob > P or max_blob < 0:
        return None
    EXTB = max(2, -(-max(max_blob, 1) * N_C // P))
    if EXTB > 4096:
        return None
    cfg = (NFG, NB4, NLAST, F, C, R, EXTB)

    b0f = np.float32(PAD_X)
    in_maps = []
    host_const = 0.0
    coef = _coef_vec(cfg)
    for c in range(N_CORES):
        vfg, vb4, vla = cores_fg[c], cores_b4[c], cores_la[c]
        fg_rows = vfg[:NFG * P]
        blob_rows = vfg[NFG * P:]
        b4_rows = vb4[:NB4 * P]
        last_rows = np.concatenate([vb4[NB4 * P:], vla])

        x_fg = np.full((NFG * P, N_C), b0f, ml_dtypes.bfloat16)
        x_fg[:len(fg_rows)] = xb[fg_rows]
        x_b4 = np.full((max(NB4, 1) * P, max(F, 1)), b0f, ml_dtypes.bfloat16)
        if NB4:
            x_b4[:len(b4_rows)] = xb[b4_rows, :F]
        x_la = np.full((NLAST * P, N_C), b0f, ml_dtypes.bfloat16)
        x_la[:len(last_rows)] = xb[last_rows]
        x_eb = np.full((P * EXTB,), b0f, ml_dtypes.bfloat16)
        if len(blob_rows):
            x_eb[:len(blob_rows) * N_C] = xb[blob_rows].reshape(-1)
        x_eb = x_eb.reshape(P, EXTB)

        # fg-path elements (pads cancel exactly); bg-path real elements
        host_const += (A0 - R_T - THR) * (NFG * P * N_C + P * EXTB)
        host_const += D0 * (len(b4_rows) * F)
        tl = t[last_rows]
        host_const += D0 * float(
            ((tl & 4) > 0).sum() * F + ((tl & 2) > 0).sum() * C
            + ((tl & 1) > 0).sum() * R)

        NGCOL = NSLOT + 1
        wa_g = np.zeros((P, NGCOL), np.float32)
        wb_g = np.zeros((P, NGCOL), np.float32)
        wg_g = np.zeros((P, NGCOL), np.float32)
        g_g = np.zeros((P, NGCOL), ml_dtypes.bfloat16)

        def fill(rows, colbase):
            for r_i, row in enumerate(rows):
                k, p = divmod(r_i, P)
                g_g[p, colbase + k] = xb[row, labp[row]]
                wa_g[p, colbase + k] = wa_all[row]
                wb_g[p, colbase + k] = wb_all[row]
                wg_g[p, colbase + k] = 1.0

        fill(fg_rows, 0)
        if NB4:
            fill(b4_rows, NFG)
        fill(last_rows, NFG + NB4)
        for r_i, row in enumerate(blob_rows):
            g_g[r_i, NSLOT] = xb[row, labp[row]]
            wa_g[r_i, NSLOT] = wa_all[row]
            wb_g[r_i, NSLOT] = wb_all[row]
            wg_g[r_i, NSLOT] = 1.0

        lind = np.zeros((P, 3 * NLAST), ml_dtypes.bfloat16)
        for r_i, row in enumerate(last_rows):
            k, p = divmod(r_i, P)
            ti = t[row]
            if ti & 4:
                lind[p, 3 * k + 0] = 1.0
            if ti & 2:
                lind[p, 3 * k + 1] = 1.0
            if ti & 1:
                lind[p, 3 * k + 2] = 1.0

        in_maps.append({
            "x_fg": x_fg, "x_b4": x_b4, "x_la": x_la, "x_eb": x_eb,
            "g": g_g, "wa": wa_g, "wb": wb_g, "wg": wg_g,
            "lind": lind, "coef": coef,
        })
    return cfg, in_maps, host_const


def kernel(cls_logits, labels, rare_mask, common_mask, freq_mask,
           rare_sel, common_sel, freq_sel, _trace=False):
    prep = _prep(cls_logits, labels, rare_mask, common_mask, freq_mask,
                 rare_sel, common_sel, freq_sel)
    if prep is None:
        return _kernel_fallback(cls_logits, labels, rare_mask, common_mask,
                                freq_mask, rare_sel, common_sel, freq_sel,
                                _trace=_trace)
    cfg, in_maps, host_const = prep
    nc = _get_nc(cfg)
    res = run_bass_kernel_spmd(nc, in_maps, core_ids=list(range(N_CORES)),
                               trace=_trace)
    total = float(host_const)
    for c in range(N_CORES):
        total += float(res.results[c]["out"].reshape(()))
    out = np.asarray(np.float32(total / N_I))
    if _trace:
        return out, res
    return out


# ---------------------------------------------------------------------------
# Fallback path (exact, baseline Exp+Ln implementation) used when the fast
# path's structural assumptions about the inputs do not hold.
# ---------------------------------------------------------------------------

K_TILES = N_LOC // P
TAU = float(math.log(1.0 + 0.7 / 0.3))
N_CHUNKS = [(0, 512), (512, 1024), (1024, N_C)]


def _build_nc_fallback():
    nc = bacc.Bacc(None, target_bir_lowering=False)
    x = nc.dram_tensor("x", [N_LOC, N_C], BF16, kind="ExternalInput")
    r_d = nc.dram_tensor("r", [P, K_TILES, 8], BF16, kind="ExternalInput")
    rp_d = nc.dram_tensor("rp", [P, K_TILES, 8], BF16, kind="ExternalInput")
    u_d = nc.dram_tensor("u", [8, N_C], BF16, kind="ExternalInput")
    uc_d = nc.dram_tensor("uc", [8, N_C], BF16, kind="ExternalInput")
    a_d = nc.dram_tensor("wa", [P, K_TILES], F32, kind="ExternalInput")
    b_d = nc.dram_tensor("wb", [P, K_TILES], F32, kind="ExternalInput")
    goff_d = nc.dram_tensor("goff", [P, K_TILES], I32, kind="ExternalInput")
    out_d = nc.dram_tensor("out", [1, 1], F32, kind="ExternalOutput")

    xv = x.rearrange("(k p) c -> p k c", p=P)
    x_flat = x.rearrange("r (c one) -> (r c) one", one=1)
    SIZES = [2] * 7 + [1, 1]
    STARTS = [sum(SIZES[:i]) for i in range(len(SIZES))]
    N_ST = len(SIZES)

    with tile.TileContext(nc) as tc, ExitStack() as ctx:
        const = ctx.enter_context(tc.tile_pool(name="const", bufs=1))
        xpool = ctx.enter_context(tc.tile_pool(name="x", bufs=1))
        epool = ctx.enter_context(tc.tile_pool(name="e", bufs=1))
        apool = ctx.enter_context(tc.tile_pool(name="a", bufs=1))
        cpool = ctx.enter_context(tc.tile_pool(name="c", bufs=1))
        mpool = ctx.enter_context(tc.tile_pool(name="m", bufs=1))
        psum = ctx.enter_context(tc.tile_pool(name="psum", bufs=1, space="PSUM"))
        fin = ctx.enter_context(tc.tile_pool(name="fin", bufs=1))

        xs_tiles = [None] * N_ST

        def load_xs(s):
            k0, sz = STARTS[s], SIZES[s]
            xs_tiles[s] = xpool.tile([P, sz, N_C], BF16, tag="xs",
                                     name=f"xs{s}", bufs=4)
            nc.sync.dma_start(xs_tiles[s][:], xv[:, k0:k0 + sz, :])

        load_xs(0)
        load_xs(1)

        r_sb = const.tile([P, K_TILES, 8], BF16)
        nc.gpsimd.dma_start(r_sb[:], r_d[:])
        rp_sb = const.tile([P, K_TILES, 8], BF16)
        nc.gpsimd.dma_start(rp_sb[:], rp_d[:])
        goff_sb = const.tile([P, K_TILES], I32)
        nc.gpsimd.dma_start(goff_sb[:], goff_d[:])
        u_sb = const.tile([8, N_C], BF16)
        nc.gpsimd.dma_start(u_sb[:], u_d[:])
        uc_sb = const.tile([8, N_C], BF16)
        nc.gpsimd.dma_start(uc_sb[:], uc_d[:])
        a_sb = const.tile([P, K_TILES], F32)
        nc.gpsimd.dma_start(a_sb[:], a_d[:])
        b_sb = const.tile([P, K_TILES], F32)
        nc.gpsimd.dma_start(b_sb[:], b_d[:])
        ones = const.tile([P, 1], F32)
        nc.vector.memset(ones[:], 1.0)

        g_sb = const.tile([P, K_TILES], BF16)
        nc.gpsimd.indirect_dma_start(
            out=g_sb[:, :], out_offset=None, in_=x_flat,
            in_offset=bass.IndirectOffsetOnAxis(ap=goff_sb[:, :], axis=0))

        p1 = psum.tile([8, N_C], F32, space="PSUM")
        p2 = psum.tile([8, N_C], F32, space="PSUM")

        eg = fin.tile([P, K_TILES], F32)
        spg = fin.tile([P, K_TILES], F32)

        act_order = []
        warm = fin.tile([1, 2], F32)
        nc.vector.memset(warm[:], 0.0)
        warm_o = fin.tile([1, 2], F32)
        act_order.append(nc.scalar.activation(warm_o[:], warm[:], AF.Exp))
        e_tiles = [None] * N_ST
        a_tiles = [None] * N_ST
        for s in range(N_ST):
            if xs_tiles[s] is None:
                load_xs(s)
            sz = SIZES[s]
            e_tiles[s] = epool.tile([P, sz, N_C], BF16, tag="e",
                                    name=f"et{s}", bufs=10)
            act_order.append(nc.scalar.activation(
                e_tiles[s][:], xs_tiles[s][:], AF.Exp))
        act_order.append(nc.scalar.activation(eg[:], g_sb[:], AF.Exp))
        act_order.append(nc.scalar.activation(spg[:], eg[:], AF.Ln, bias=1.0))
        for s in range(N_ST):
            sz = SIZES[s]
            a_tiles[s] = apool.tile([P, sz, N_C], BF16, tag="a",
                                    name=f"at{s}", bufs=4)
            act_order.append(nc.scalar.activation(
                a_tiles[s][:], e_tiles[s][:], AF.Ln, bias=1.0))
        for s in range(N_ST):
            sz = SIZES[s]
            a_t = a_tiles[s]
            c_t = cpool.tile([P, sz, N_C], BF16, tag="c", name=f"ct{s}", bufs=3)
            nc.vector.tensor_scalar(c_t[:], a_t[:], TAU, None, OP.is_ge)
            m_t = mpool.tile([P, sz, N_C], BF16, tag="m", name=f"mt{s}", bufs=3)
            nc.vector.tensor_tensor(m_t[:], c_t[:], a_t[:], OP.mult)
            for j in range(sz):
                k = STARTS[s] + j
                for n0, n1 in N_CHUNKS:
                    nc.tensor.matmul(
                        p1[:, n0:n1], r_sb[:, k, :], a_t[:, j, n0:n1],
                        start=(k == 0), stop=(k == K_TILES - 1))
            for j in range(sz):
                k = STARTS[s] + j
                for n0, n1 in N_CHUNKS:
                    nc.tensor.matmul(
                        p2[:, n0:n1], rp_sb[:, k, :], m_t[:, j, n0:n1],
                        start=(k == 0), stop=(k == K_TILES - 1))

        for prev, nxt in zip(act_order, act_order[1:]):
            tile.add_dep_helper(nxt.ins, prev.ins, sync=False,
                                reason="ACT table-load grouping")

        t1 = fin.tile([8, N_C], BF16)
        nc.vector.tensor_tensor(t1[:], p1[:], u_sb[:], OP.mult)
        t2 = fin.tile([8, N_C], BF16)
        nc.vector.tensor_tensor(t2[:], p2[:], uc_sb[:], OP.mult)
        t3 = fin.tile([8, N_C], BF16)
        nc.vector.tensor_tensor(t3[:], t1[:], t2[:], OP.add)
        r8 = fin.tile([8, 1], F32)
        nc.vector.reduce_sum(r8[:], t3[:], axis=mybir.AxisListType.X)

        g32 = fin.tile([P, K_TILES], F32)
        nc.vector.tensor_copy(g32[:], g_sb[:])
        mlt = fin.tile([P, K_TILES], F32)
        nc.vector.tensor_scalar(mlt[:], g32[:], THR, None, OP.is_lt)
        w1 = fin.tile([P, K_TILES], F32)
        nc.vector.tensor_tensor(w1[:], mlt[:], b_sb[:], OP.mult)
        w2 = fin.tile([P, K_TILES], F32)
        nc.vector.tensor_tensor(w2[:], w1[:], a_sb[:], OP.add)
        t4 = fin.tile([P, K_TILES], F32)
        nc.vector.tensor_tensor(t4[:], w2[:], spg[:], OP.mult)
        t5 = fin.tile([P, K_TILES], F32)
        nc.vector.tensor_tensor(t5[:], t4[:], g32[:], OP.subtract)
        rr = fin.tile([P, 1], F32)
        nc.vector.reduce_sum(rr[:], t5[:], axis=mybir.AxisListType.X)

        s_ps = psum.tile([1, 1], F32, space="PSUM")
        nc.tensor.matmul(s_ps[:], ones[:], rr[:], start=True, stop=False,
                         skip_group_check=True)
        nc.tensor.matmul(s_ps[:], ones[:8, :], r8[:], start=False, stop=True,
                         skip_group_check=True)
        out_sb = fin.tile([1, 1], F32)
        nc.vector.tensor_copy(out_sb[:], s_ps[:])
        nc.sync.dma_start(out_d[:], out_sb[:])

    nc.finalize()
    return nc


def _prep_fallback(cls_logits, labels, rare_mask, common_mask, freq_mask,
                   rare_sel, common_sel, freq_sel):
    x = np.ascontiguousarray(
        np.asarray(cls_logits, dtype=np.float32).astype(ml_dtypes.bfloat16))
    lab = np.asarray(labels).astype(np.int64)
    rm = np.asarray(rare_mask).astype(np.float32)
    cm = np.asarray(common_mask).astype(np.float32)
    fm = np.asarray(freq_mask).astype(np.float32)
    rs = np.asarray(rare_sel).astype(np.int64)
    cs = np.asarray(common_sel).astype(np.int64)
    fs = np.asarray(freq_sel).astype(np.int64)

    t = rs + 2 * cs + 4 * fs
    fgv = (lab != 0).astype(np.float32)
    Rm = np.zeros((N_I, 8), np.float32)
    Rm[np.arange(N_I), t] = 1.0
    Rp = Rm * fgv[:, None]

    u8 = np.zeros((8, N_C), np.float32)
    for tt_ in range(8):
        m = np.zeros(N_C, np.float32)
        if tt_ & 1:
            m = np.maximum(m, rm)
        if tt_ & 2:
            m = np.maximum(m, cm)
        if tt_ & 4:
            m = np.maximum(m, fm)
        u8[tt_] = m

    h = u8[t, lab]
    wa = (1.0 - h) * (1.0 - fgv)
    wb = (1.0 - h) * fgv

    loc = np.arange(N_LOC, dtype=np.int64)

    def fold(v):
        return np.ascontiguousarray(v.reshape(K_TILES, P).T)

    in_maps = []
    for c in range(N_CORES):
        rows = slice(c * N_LOC, (c + 1) * N_LOC)
        goff = loc * N_C + lab[rows]
        in_maps.append({
            "x": x[rows],
            "r": np.ascontiguousarray(
                Rm[rows].reshape(K_TILES, P, 8).transpose(1, 0, 2)
            ).astype(ml_dtypes.bfloat16),
            "rp": np.ascontiguousarray(
                Rp[rows].reshape(K_TILES, P, 8).transpose(1, 0, 2)
            ).astype(ml_dtypes.bfloat16),
            "u": u8.astype(ml_dtypes.bfloat16),
            "uc": np.ascontiguousarray(1.0 - u8).astype(ml_dtypes.bfloat16),
            "wa": fold(wa[rows].astype(np.float32)),
            "wb": fold(wb[rows].astype(np.float32)),
            "goff": fold(goff).astype(np.int32),
        })
    return in_maps


_NC_FALLBACK = None


def _kernel_fallback(cls_logits, labels, rare_mask, common_mask, freq_mask,
                     rare_sel, common_sel, freq_sel, _trace=False):
    global _NC_FALLBACK
    in_maps = _prep_fallback(cls_logits, labels, rare_mask, common_mask,
                             freq_mask, rare_sel, common_sel, freq_sel)
    if _NC_FALLBACK is None:
        _NC_FALLBACK = _build_nc_fallback()
    res = run_bass_kernel_spmd(_NC_FALLBACK, in_maps,
                               core_ids=list(range(N_CORES)), trace=_trace)
    total = np.float32(0.0)
    for c in range(N_CORES):
        total += res.results[c]["out"].reshape(())
    out = np.asarray(total / np.float32(N_I), dtype=np.float32)
    if _trace:
        return out, res
    return out
